# revision 1
# baseline (speedup 1.0000x reference)
"""TRN2 Bass kernel for nn_IrrepsLinear (e3nn-style per-irrep linear layer).

Computation (per node n, N=200000 nodes, 480 features):
  out0 = (x0 @ W0 + modal_attr[batch[n]] @ W0m) / sqrt(130)   cols   0:128
  out1 = einsum('nim,io->nom', x1, W1) / sqrt(64)             cols 128:320
  out2 = einsum('nim,io->nom', x2, W2) / sqrt(32)             cols 320:480

Strategy: data-parallel over nodes across 8 NeuronCores (25000 nodes/core,
padded to 25088 = 49 macro-tiles of 512). The per-irrep einsum is folded into
one 480x480 block-diagonal weight Wfull (scales included), so the whole layer
is out = x @ Wfull plus a per-node modal gather realized on-device as a
one-hot matmul:
  modal contribution = onehot(batch) @ G,   G = modal_attr @ (W0m/sqrt(130))

Per-core kernel structure (per 512-node macro tile, nodes on PSUM partitions):
  - x shard stored node-block-major [128, nb*480] (host layout prep) so every
    DMA row is contiguous; loaded via SWDGE casting DMA fp32->float32r
    (float32r: TF32-like 1cyc/row matmul dtype, ~1.6e-4 rel rounding)
  - PE transposes x block [128n, 128f] -> psum [128f, 128n] (f32r, 1.5cyc/row)
  - ACT copies psum -> SBUF (xT operand tiles)
  - main matmuls: lhsT = xT chunk (f32r), rhs = padded Wfull blocks (f32r),
    accumulated into psum_o [128 nodes, 480] per 128-node block
  - modal: batch ids broadcast via fp16 ones-matmul (exact for ids<2048),
    one-hot chunks via DVE is_equal, 4 bf16 matmuls add G[batch] to cols 0:128
  - DVE/ACT copy psum_o -> out SBUF, output DMAed back in the same
    node-block-major layout and unpermuted on host
"""
import numpy as np

import concourse.bass as bass
import concourse.mybir as mybir
import concourse.tile as tile
from concourse import bacc
from concourse.bass_utils import run_bass_kernel_spmd
from concourse.masks import make_identity

f32 = mybir.dt.float32
f32r = mybir.dt.float32r
bf16 = mybir.dt.bfloat16
f16 = mybir.dt.float16
i32 = mybir.dt.int32

N_CORES = 8
MUL0, MUL1, MUL2, NMOD = 128, 64, 32, 2
DTOT = 480
NODES_PER_MACRO = 512
NB = 4  # 128-node blocks per macro

CHUNKS = [(0, 128), (128, 256), (256, 384), (384, 480)]
# rhs blocks: (row range = f_in chunk, col range = padded f_out window).
# PSUM discipline: first matmul (start=True) covers cols 128:480; the modal
# matmuls then overwrite the still-fresh cols 0:128; the rest accumulate.
# Col windows padded to >=256 so float32r streams at 1 cyc/row.
RBLK = [((256, 384), (128, 480)),
        ((0, 128), (0, 256)),
        ((128, 256), (128, 384)),
        ((384, 480), (224, 480))]


def _build_wfull(W0, W1, W2):
    inv0 = np.float32(1.0) / np.sqrt(np.float32(MUL0 + NMOD))
    inv1 = np.float32(1.0) / np.sqrt(np.float32(MUL1))
    inv2 = np.float32(1.0) / np.sqrt(np.float32(MUL2))
    Wfull = np.zeros((DTOT, DTOT), dtype=np.float32)
    Wfull[0:128, 0:128] = W0 * inv0
    for m in range(3):
        Wfull[128 + m:320:3, 128 + m:320:3] = W1 * inv1
    for m in range(5):
        Wfull[320 + m:480:5, 320 + m:480:5] = W2 * inv2
    return Wfull


def _host_prep(x, modal_attr, W0, W0m, W1, W2, batch):
    x = np.ascontiguousarray(np.asarray(x, dtype=np.float32))
    N = x.shape[0]
    ns = N // N_CORES
    ns_pad = ((ns + NODES_PER_MACRO - 1) // NODES_PER_MACRO) * NODES_PER_MACRO
    Wfull = _build_wfull(np.asarray(W0, dtype=np.float32),
                         np.asarray(W1, dtype=np.float32),
                         np.asarray(W2, dtype=np.float32))
    inv0 = np.float32(1.0) / np.sqrt(np.float32(MUL0 + NMOD))
    w0m_s = (np.asarray(W0m, dtype=np.float32) * inv0).astype(np.float32)
    rblks = [np.ascontiguousarray(Wfull[r0:r1, c0:c1])
             for (r0, r1), (c0, c1) in RBLK]
    batch = np.asarray(batch)
    mattr = np.ascontiguousarray(np.asarray(modal_attr, dtype=np.float32))
    in_maps = []
    for i in range(N_CORES):
        xs = np.zeros((ns_pad, DTOT), dtype=np.float32)
        xs[:ns] = x[i * ns:(i + 1) * ns]
        # node-block-major: partition p holds nodes {128*nb + p} contiguously
        xs2 = np.ascontiguousarray(
            xs.reshape(-1, 128, DTOT).transpose(1, 0, 2).reshape(128, -1))
        bs = np.zeros((ns_pad,), dtype=np.float16)
        bs[:ns] = batch[i * ns:(i + 1) * ns].astype(np.float16)
        in_maps.append({
            "xs": xs2, "bs": bs,
            "r0": rblks[0], "r1": rblks[1], "r2": rblks[2], "r3": rblks[3],
            "w0m": w0m_s, "mattr": mattr,
        })
    return in_maps, ns, ns_pad


def _build_nc(ns_pad, macros_per_super=4):
    assert ns_pad % NODES_PER_MACRO == 0
    nmacro = ns_pad // NODES_PER_MACRO
    supers = []
    m0 = 0
    while m0 < nmacro:
        msup = min(macros_per_super, nmacro - m0)
        supers.append((m0, msup))
        m0 += msup
    nb_per_super = NB * macros_per_super
    nc = bacc.Bacc("TRN2", target_bir_lowering=False, debug=False)

    nbs_tot = ns_pad // 128
    xs = nc.dram_tensor("xs", [128, nbs_tot * DTOT], f32,
                        kind="ExternalInput").ap()
    bs = nc.dram_tensor("bs", [ns_pad], f16, kind="ExternalInput").ap()
    rdr = []
    for k, ((r0_, r1_), (c0_, c1_)) in enumerate(RBLK):
        rdr.append(nc.dram_tensor(f"r{k}", [r1_ - r0_, c1_ - c0_], f32,
                                  kind="ExternalInput").ap())
    w0m = nc.dram_tensor("w0m", [NMOD, 128], f32, kind="ExternalInput").ap()
    mattr = nc.dram_tensor("mattr", [512, NMOD], f32, kind="ExternalInput").ap()
    ys = nc.dram_tensor("ys", [128, nbs_tot * DTOT], f32,
                        kind="ExternalOutput").ap()

    with tile.TileContext(nc) as tc:
        with tc.tile_pool(name="const", bufs=1) as cpool, \
             tc.tile_pool(name="sb", bufs=3) as sb, \
             tc.tile_pool(name="sbx", bufs=3) as sbx, \
             tc.tile_pool(name="sbo", bufs=2) as sbo, \
             tc.tile_pool(name="psb", bufs=2, space="PSUM") as psb, \
             tc.tile_pool(name="ps3", bufs=3, space="PSUM") as ps3:

            # ---------------- setup ----------------
            ident = cpool.tile([128, 128], f32, tag="ident")
            make_identity(nc, ident[:])
            ident_fr = cpool.tile([128, 128], f32r, tag="identfr")
            nc.vector.tensor_copy(ident_fr[:], ident[:])

            ones16 = cpool.tile([1, 128], f16, tag="ones16")
            nc.gpsimd.memset(ones16[:], 1.0)

            pidxf = []
            for c in range(4):
                t_i = cpool.tile([128, 1], i32, tag=f"pidx{c}i")
                nc.gpsimd.iota(t_i[:], pattern=[[0, 1]], base=128 * c,
                               channel_multiplier=1)
                t_f = cpool.tile([128, 1], f32, tag=f"pidx{c}f")
                nc.vector.tensor_copy(t_f[:], t_i[:])
                pidxf.append(t_f)

            rfr = []
            for k in range(4):
                rows, cols = rdr[k].shape
                t0 = cpool.tile([rows, cols], f32, tag=f"r{k}raw")
                nc.sync.dma_start(out=t0[:], in_=rdr[k])
                t1 = cpool.tile([rows, cols], f32r, tag=f"r{k}fr")
                nc.vector.tensor_copy(t1[:], t0[:])
                rfr.append(t1)

            w0m_sb = cpool.tile([NMOD, 128], f32, tag="w0mraw")
            nc.sync.dma_start(out=w0m_sb[:], in_=w0m)
            w0m_fr = cpool.tile([NMOD, 128], f32r, tag="w0mfr")
            nc.vector.tensor_copy(w0m_fr[:], w0m_sb[:])

            mattr_sb = cpool.tile([128, 4 * NMOD], f32, tag="mattrraw")
            nc.sync.dma_start(out=mattr_sb[:].rearrange("g (c j) -> g c j", c=4),
                              in_=mattr.rearrange("(c g) j -> g c j", c=4))
            ps_mat = ps3.tile([NMOD, 512], f32, tag="xt")
            for c in range(4):
                nc.tensor.matmul(ps_mat[:, 128 * c:128 * (c + 1)],
                                 mattr_sb[:, NMOD * c:NMOD * (c + 1)],
                                 ident[:], is_transpose=True,
                                 start=(c == 0), stop=(c == 3),
                                 skip_group_check=True)
            maT_fr = cpool.tile([NMOD, 512], f32r, tag="maTfr")
            nc.vector.tensor_copy(maT_fr[:], ps_mat[:])
            gch = []
            for c in range(4):
                ps_g = ps3.tile([128, 128], f32, tag="po")
                nc.tensor.matmul(ps_g[:], maT_fr[:, 128 * c:128 * (c + 1)],
                                 w0m_fr[:], start=True, stop=True)
                g_c = cpool.tile([128, 128], bf16, tag=f"g{c}")
                nc.vector.tensor_copy(g_c[:], ps_g[:])
                gch.append(g_c)

            # ---------------- main loop ----------------
            for m0, msup in supers:
                n0 = m0 * NODES_PER_MACRO
                nbs = NB * msup
                x_sb = sbx.tile([128, nb_per_super * DTOT], f32r, tag="x")
                h1 = (msup + 1) // 2
                col0 = m0 * NB * DTOT
                for lo, hi in ((0, h1), (h1, msup)):
                    if hi <= lo:
                        continue
                    # SWDGE casting DMA: fp32 HBM -> float32r SBUF
                    nc.gpsimd.dma_start(
                        out=x_sb[:, lo * NB * DTOT:hi * NB * DTOT],
                        in_=xs[:, col0 + lo * NB * DTOT:col0 + hi * NB * DTOT])

                bs_sb = sb.tile([1, NODES_PER_MACRO * macros_per_super], f16,
                                tag="bs")
                nodes = msup * NODES_PER_MACRO
                nc.sync.dma_start(
                    out=bs_sb[:, :nodes],
                    in_=bs[n0:n0 + nodes].rearrange("(one n) -> one n", one=1))

                out_sb = sbo.tile([128, nb_per_super * DTOT], f32, tag="out")

                for q in range(msup):
                    ps_bb = psb.tile([128, NODES_PER_MACRO], f32, tag="bb")
                    nc.tensor.matmul(
                        ps_bb[:], ones16[:],
                        bs_sb[:, NODES_PER_MACRO * q:NODES_PER_MACRO * (q + 1)],
                        start=True, stop=True)
                    bb_sb = sb.tile([128, NODES_PER_MACRO], f32, tag="bbs")
                    nc.vector.tensor_copy(bb_sb[:], ps_bb[:])

                    ohs = []
                    for c in range(4):
                        oh = sb.tile([128, NODES_PER_MACRO], bf16, tag=f"oh{c}")
                        nc.vector.tensor_scalar(oh[:], bb_sb[:], pidxf[c][:],
                                                None,
                                                op0=mybir.AluOpType.is_equal)
                        ohs.append(oh)

                    for nbq in range(NB):
                        nb = NB * q + nbq
                        ps_xt = ps3.tile([128, 512], f32r, tag="xt")
                        for c, (f0, f1) in enumerate(CHUNKS):
                            cp = f1 - f0
                            nc.tensor.matmul(
                                ps_xt[0:cp, 128 * c:128 * c + 128],
                                x_sb[:, DTOT * nb + f0:DTOT * nb + f1],
                                ident_fr[:], is_transpose=True,
                                start=(c == 0), stop=(c == 3),
                                skip_group_check=True)
                        xt_sb = sb.tile([128, 512], f32r, tag="xts")
                        nc.scalar.copy(xt_sb[:, 0:384], ps_xt[:, 0:384])
                        nc.scalar.copy(xt_sb[0:96, 384:512],
                                       ps_xt[0:96, 384:512])

                        ps_o = ps3.tile([128, DTOT], f32, tag="po")

                        def mm_rblk(k, start, stop):
                            (r0_, r1_), (c0_, c1_) = RBLK[k]
                            kp = r1_ - r0_
                            ch = r0_ // 128
                            nc.tensor.matmul(
                                ps_o[:, c0_:c1_],
                                xt_sb[0:kp, 128 * ch:128 * ch + 128],
                                rfr[k][:], start=start, stop=stop,
                                skip_group_check=True)

                        mm_rblk(0, True, False)
                        for c in range(4):
                            nc.tensor.matmul(
                                ps_o[:, 0:128],
                                ohs[c][:, 128 * nbq:128 * nbq + 128],
                                gch[c][:], start=False, stop=False,
                                skip_group_check=True)
                        mm_rblk(1, False, False)
                        mm_rblk(2, False, False)
                        mm_rblk(3, False, True)

                        if nbq == 3:
                            nc.scalar.copy(
                                out_sb[:, DTOT * nb:DTOT * (nb + 1)], ps_o[:])
                        else:
                            nc.vector.tensor_copy(
                                out_sb[:, DTOT * nb:DTOT * (nb + 1)], ps_o[:])

                for lo, hi in ((0, h1), (h1, msup)):
                    if hi <= lo:
                        continue
                    nc.sync.dma_start(
                        out=ys[:, col0 + lo * NB * DTOT:col0 + hi * NB * DTOT],
                        in_=out_sb[:, lo * NB * DTOT:hi * NB * DTOT])

    nc.compile()
    return nc


_NC_CACHE = {}


def kernel(x, modal_attr, W0, W0m, W1, W2, batch):
    in_maps, ns, ns_pad = _host_prep(x, modal_attr, W0, W0m, W1, W2, batch)
    if ns_pad not in _NC_CACHE:
        _NC_CACHE[ns_pad] = _build_nc(ns_pad)
    nc = _NC_CACHE[ns_pad]
    res = run_bass_kernel_spmd(nc, in_maps, core_ids=list(range(N_CORES)))
    nbs = ns_pad // 128
    outs = []
    for i in range(N_CORES):
        ys2 = res.results[i]["ys"]
        outs.append(ys2.reshape(128, nbs, DTOT).transpose(1, 0, 2)
                    .reshape(ns_pad, DTOT)[:ns])
    return np.ascontiguousarray(np.concatenate(outs, axis=0))


# revision 2
# speedup vs baseline: 1.4390x; 1.4390x over previous
"""TRN2 Bass kernel for nn_IrrepsLinear (e3nn-style per-irrep linear layer).

Computation (per node n, N=200000 nodes, 480 features):
  out0 = (x0 @ W0 + modal_attr[batch[n]] @ W0m) / sqrt(130)   cols   0:128
  out1 = einsum('nim,io->nom', x1, W1) / sqrt(64)             cols 128:320
  out2 = einsum('nim,io->nom', x2, W2) / sqrt(32)             cols 320:480

Strategy: data-parallel over nodes across 8 NeuronCores (25000 nodes/core,
padded to 25088 = 49 macro-tiles of 512 nodes). The per-irrep einsums fold
into one 480x480 block-diagonal weight Wfull (scales included), so the layer
is out = x @ Wfull plus a per-node modal gather realized on-device as a
one-hot matmul:
  modal contribution = onehot(batch) @ G,   G = modal_attr @ (W0m/sqrt(130))

The pipeline computes in fp16 (11-bit mantissa, ~5e-4 relative error - the
tensor engine runs 16-bit operands at 1 cycle/row vs 4 for fp32):
  - x shards are cast to fp16 and stored node-block-major [128, nb*480] on
    the host so every DMA row is a contiguous DRAM run
  - x loads ride the SWDGE queue, output stores the HWDGE queue (two DMA
    queues running concurrently)
  - PE transposes x blocks [128n, 128f] -> psum [128f, 128n] (fp16 1cyc/row)
  - ACT copies psum -> SBUF xT operand tiles
  - matmuls: lhsT = xT chunk, rhs = Wfull blocks (fp16), accumulated into
    psum_o [128 nodes, 480] fp32 per node-block; PSUM freshness discipline:
    R2 (start=True, cols 128:480) -> 4 modal one-hot matmuls (cols 0:128,
    fresh overwrite) -> R0/R1/R3 accumulate
  - modal one-hot: batch ids broadcast via fp16 ones-matmul (exact for
    ids < 2048), 4 chunk compares on DVE (fp16 in, fp32 per-partition iota
    scalar), bf16/fp16 G matmuls
  - psum_o copied to an fp16 out tile (DVE + ACT), stored fp16, upcast on
    host during the inverse layout permutation
"""
import numpy as np

import concourse.bass as bass
import concourse.mybir as mybir
import concourse.tile as tile
from concourse import bacc
from concourse.bass_utils import run_bass_kernel_spmd
from concourse.masks import make_identity

f32 = mybir.dt.float32
f16 = mybir.dt.float16
i32 = mybir.dt.int32

N_CORES = 8
MUL0, MUL1, MUL2, NMOD = 128, 64, 32, 2
DTOT = 480
NODES_PER_MACRO = 512
NB = 4  # 128-node blocks per macro

CHUNKS = [(0, 128), (128, 256), (256, 384), (384, 480)]
# rhs blocks: (row range = f_in chunk, col range = f_out window)
RBLK = [((256, 384), (128, 480)),
        ((0, 128), (0, 128)),
        ((128, 256), (128, 320)),
        ((384, 480), (320, 480))]


def _build_wfull(W0, W1, W2):
    inv0 = np.float32(1.0) / np.sqrt(np.float32(MUL0 + NMOD))
    inv1 = np.float32(1.0) / np.sqrt(np.float32(MUL1))
    inv2 = np.float32(1.0) / np.sqrt(np.float32(MUL2))
    Wfull = np.zeros((DTOT, DTOT), dtype=np.float32)
    Wfull[0:128, 0:128] = W0 * inv0
    for m in range(3):
        Wfull[128 + m:320:3, 128 + m:320:3] = W1 * inv1
    for m in range(5):
        Wfull[320 + m:480:5, 320 + m:480:5] = W2 * inv2
    return Wfull


def _host_prep(x, modal_attr, W0, W0m, W1, W2, batch):
    x = np.asarray(x)
    N = x.shape[0]
    ns = N // N_CORES
    ns_pad = ((ns + NODES_PER_MACRO - 1) // NODES_PER_MACRO) * NODES_PER_MACRO
    Wfull = _build_wfull(np.asarray(W0, dtype=np.float32),
                         np.asarray(W1, dtype=np.float32),
                         np.asarray(W2, dtype=np.float32))
    inv0 = np.float32(1.0) / np.sqrt(np.float32(MUL0 + NMOD))
    w0m_s = (np.asarray(W0m, dtype=np.float32) * inv0).astype(np.float32)
    rblks = [np.ascontiguousarray(Wfull[r0:r1, c0:c1])
             for (r0, r1), (c0, c1) in RBLK]
    batch = np.asarray(batch)
    mattr = np.ascontiguousarray(np.asarray(modal_attr, dtype=np.float32))
    in_maps = []
    for i in range(N_CORES):
        xs = np.zeros((ns_pad, DTOT), dtype=np.float16)
        xs[:ns] = x[i * ns:(i + 1) * ns].astype(np.float16)
        # node-block-major: partition p holds nodes {128*nb + p} contiguously
        xs2 = np.ascontiguousarray(
            xs.reshape(-1, 128, DTOT).transpose(1, 0, 2).reshape(128, -1))
        bs = np.zeros((ns_pad,), dtype=np.float16)
        bs[:ns] = batch[i * ns:(i + 1) * ns].astype(np.float16)
        in_maps.append({
            "xs": xs2, "bs": bs,
            "r0": rblks[0], "r1": rblks[1], "r2": rblks[2], "r3": rblks[3],
            "w0m": w0m_s, "mattr": mattr,
        })
    return in_maps, ns, ns_pad


def _build_nc(ns_pad, macros_per_super=4):
    assert ns_pad % NODES_PER_MACRO == 0
    nmacro = ns_pad // NODES_PER_MACRO
    supers = []
    m0 = 0
    while m0 < nmacro:
        msup = min(macros_per_super, nmacro - m0)
        supers.append((m0, msup))
        m0 += msup
    nb_per_super = NB * macros_per_super
    nc = bacc.Bacc("TRN2", target_bir_lowering=False, debug=False)

    nbs_tot = ns_pad // 128
    xs = nc.dram_tensor("xs", [128, nbs_tot * DTOT], f16,
                        kind="ExternalInput").ap()
    bs = nc.dram_tensor("bs", [ns_pad], f16, kind="ExternalInput").ap()
    rdr = []
    for k, ((r0_, r1_), (c0_, c1_)) in enumerate(RBLK):
        rdr.append(nc.dram_tensor(f"r{k}", [r1_ - r0_, c1_ - c0_], f32,
                                  kind="ExternalInput").ap())
    w0m = nc.dram_tensor("w0m", [NMOD, 128], f32, kind="ExternalInput").ap()
    mattr = nc.dram_tensor("mattr", [512, NMOD], f32, kind="ExternalInput").ap()
    ys = nc.dram_tensor("ys", [128, nbs_tot * DTOT], f16,
                        kind="ExternalOutput").ap()

    with tile.TileContext(nc) as tc:
        with tc.tile_pool(name="const", bufs=1) as cpool, \
             tc.tile_pool(name="sb", bufs=3) as sb, \
             tc.tile_pool(name="sbx", bufs=4) as sbx, \
             tc.tile_pool(name="sbo", bufs=3) as sbo, \
             tc.tile_pool(name="psb", bufs=2, space="PSUM") as psb, \
             tc.tile_pool(name="ps3", bufs=3, space="PSUM") as ps3:

            # ---------------- setup ----------------
            ident = cpool.tile([128, 128], f32, tag="ident")
            make_identity(nc, ident[:])
            ident_16 = cpool.tile([128, 128], f16, tag="ident16")
            nc.vector.tensor_copy(ident_16[:], ident[:])

            ones16 = cpool.tile([1, 128], f16, tag="ones16")
            nc.gpsimd.memset(ones16[:], 1.0)

            pidxf = []
            for c in range(4):
                t_i = cpool.tile([128, 1], i32, tag=f"pidx{c}i")
                nc.gpsimd.iota(t_i[:], pattern=[[0, 1]], base=128 * c,
                               channel_multiplier=1)
                t_f = cpool.tile([128, 1], f32, tag=f"pidx{c}f")
                nc.vector.tensor_copy(t_f[:], t_i[:])
                pidxf.append(t_f)

            rfr = []
            for k in range(4):
                rows, cols = rdr[k].shape
                t0 = cpool.tile([rows, cols], f32, tag=f"r{k}raw")
                nc.sync.dma_start(out=t0[:], in_=rdr[k])
                t1 = cpool.tile([rows, cols], f16, tag=f"r{k}16")
                nc.vector.tensor_copy(t1[:], t0[:])
                rfr.append(t1)

            w0m_sb = cpool.tile([NMOD, 128], f32, tag="w0mraw")
            nc.sync.dma_start(out=w0m_sb[:], in_=w0m)
            w0m_16 = cpool.tile([NMOD, 128], f16, tag="w0m16")
            nc.vector.tensor_copy(w0m_16[:], w0m_sb[:])

            mattr_sb = cpool.tile([128, 4 * NMOD], f32, tag="mattrraw")
            nc.sync.dma_start(out=mattr_sb[:].rearrange("g (c j) -> g c j", c=4),
                              in_=mattr.rearrange("(c g) j -> g c j", c=4))
            ps_mat = ps3.tile([NMOD, 512], f32, tag="xt")
            for c in range(4):
                nc.tensor.matmul(ps_mat[:, 128 * c:128 * (c + 1)],
                                 mattr_sb[:, NMOD * c:NMOD * (c + 1)],
                                 ident[:], is_transpose=True,
                                 start=(c == 0), stop=(c == 3),
                                 skip_group_check=True)
            maT_16 = cpool.tile([NMOD, 512], f16, tag="maT16")
            nc.vector.tensor_copy(maT_16[:], ps_mat[:])
            gch = []
            for c in range(4):
                ps_g = ps3.tile([128, 128], f32, tag="po")
                nc.tensor.matmul(ps_g[:], maT_16[:, 128 * c:128 * (c + 1)],
                                 w0m_16[:], start=True, stop=True)
                g_c = cpool.tile([128, 128], f16, tag=f"g{c}")
                nc.vector.tensor_copy(g_c[:], ps_g[:])
                gch.append(g_c)

            # ---------------- main loop ----------------
            for m0, msup in supers:
                n0 = m0 * NODES_PER_MACRO
                x_sb = sbx.tile([128, nb_per_super * DTOT], f16, tag="x")
                h1 = (msup + 1) // 2
                col0 = m0 * NB * DTOT
                for lo, hi in ((0, h1), (h1, msup)):
                    if hi <= lo:
                        continue
                    # x loads on the SWDGE queue (output uses the HWDGE queue)
                    nc.gpsimd.dma_start(
                        out=x_sb[:, lo * NB * DTOT:hi * NB * DTOT],
                        in_=xs[:, col0 + lo * NB * DTOT:col0 + hi * NB * DTOT])

                bs_sb = sb.tile([1, NODES_PER_MACRO * macros_per_super], f16,
                                tag="bs")
                nodes = msup * NODES_PER_MACRO
                nc.sync.dma_start(
                    out=bs_sb[:, :nodes],
                    in_=bs[n0:n0 + nodes].rearrange("(one n) -> one n", one=1))

                out_sb = sbo.tile([128, nb_per_super * DTOT], f16, tag="out")

                for q in range(msup):
                    ps_bb = psb.tile([128, NODES_PER_MACRO], f32, tag="bb")
                    nc.tensor.matmul(
                        ps_bb[:], ones16[:],
                        bs_sb[:, NODES_PER_MACRO * q:NODES_PER_MACRO * (q + 1)],
                        start=True, stop=True)
                    bb_sb = sb.tile([128, NODES_PER_MACRO], f16, tag="bbs")
                    nc.vector.tensor_copy(bb_sb[:], ps_bb[:])

                    ohs = []
                    for c in range(4):
                        oh = sb.tile([128, NODES_PER_MACRO], f16, tag=f"oh{c}")
                        nc.vector.tensor_scalar(oh[:], bb_sb[:], pidxf[c][:],
                                                None,
                                                op0=mybir.AluOpType.is_equal)
                        ohs.append(oh)

                    for nbq in range(NB):
                        nb = NB * q + nbq
                        ps_xt = ps3.tile([128, 512], f16, tag="xt")
                        for c, (f0, f1) in enumerate(CHUNKS):
                            cp = f1 - f0
                            nc.tensor.matmul(
                                ps_xt[0:cp, 128 * c:128 * c + 128],
                                x_sb[:, DTOT * nb + f0:DTOT * nb + f1],
                                ident_16[:], is_transpose=True,
                                start=(c == 0), stop=(c == 3),
                                skip_group_check=True)
                        xt_sb = sb.tile([128, 512], f16, tag="xts")
                        nc.scalar.copy(xt_sb[:, 0:384], ps_xt[:, 0:384])
                        nc.scalar.copy(xt_sb[0:96, 384:512],
                                       ps_xt[0:96, 384:512])

                        ps_o = ps3.tile([128, DTOT], f32, tag="po")

                        def mm_rblk(k, start, stop):
                            (r0_, r1_), (c0_, c1_) = RBLK[k]
                            kp = r1_ - r0_
                            ch = r0_ // 128
                            nc.tensor.matmul(
                                ps_o[:, c0_:c1_],
                                xt_sb[0:kp, 128 * ch:128 * ch + 128],
                                rfr[k][:], start=start, stop=stop,
                                skip_group_check=True)

                        mm_rblk(0, True, False)
                        for c in range(4):
                            nc.tensor.matmul(
                                ps_o[:, 0:128],
                                ohs[c][:, 128 * nbq:128 * nbq + 128],
                                gch[c][:], start=False, stop=False,
                                skip_group_check=True)
                        mm_rblk(1, False, False)
                        mm_rblk(2, False, False)
                        mm_rblk(3, False, True)

                        if nbq == 3:
                            nc.scalar.copy(
                                out_sb[:, DTOT * nb:DTOT * (nb + 1)], ps_o[:])
                        else:
                            nc.vector.tensor_copy(
                                out_sb[:, DTOT * nb:DTOT * (nb + 1)], ps_o[:])

                for lo, hi in ((0, h1), (h1, msup)):
                    if hi <= lo:
                        continue
                    nc.sync.dma_start(
                        out=ys[:, col0 + lo * NB * DTOT:col0 + hi * NB * DTOT],
                        in_=out_sb[:, lo * NB * DTOT:hi * NB * DTOT])

    nc.compile()
    return nc


_NC_CACHE = {}


def kernel(x, modal_attr, W0, W0m, W1, W2, batch):
    in_maps, ns, ns_pad = _host_prep(x, modal_attr, W0, W0m, W1, W2, batch)
    if ns_pad not in _NC_CACHE:
        _NC_CACHE[ns_pad] = _build_nc(ns_pad)
    nc = _NC_CACHE[ns_pad]
    res = run_bass_kernel_spmd(nc, in_maps, core_ids=list(range(N_CORES)))
    nbs = ns_pad // 128
    outs = []
    for i in range(N_CORES):
        ys2 = res.results[i]["ys"]
        outs.append(ys2.reshape(128, nbs, DTOT).transpose(1, 0, 2)
                    .reshape(ns_pad, DTOT)[:ns].astype(np.float32))
    return np.ascontiguousarray(np.concatenate(outs, axis=0))
